# revision 1
# baseline (speedup 1.0000x reference)
"""Group-limited MoE router kernel for Trainium2 (Bass/Tile), 8-core SPMD.

Implements, per token (row of 256 experts):
  scores = sigmoid(logits); biased = scores + bias
  group_score[g] = top2sum(biased[g*32:(g+1)*32]) for 8 groups
  keep top-4 groups, mask the rest to -inf
  topk_ids = top-8 of masked biased (descending)
  weights  = scores[topk_ids]; renormalize to sum 1; * 2.5

Data-parallel over tokens: 131072 tokens -> 8 cores x 16384.
Layout: tokens on SBUF partitions (128/slab), experts on the free dim.
"""

import numpy as np

TOKENS = 131072
E = 256
G = 8
EPG = 32
K = 8
KG = 4
SCALE = 2.5
N_CORES = 8
TPC = TOKENS // N_CORES

NEG = -1.0e30  # group mask value


def build_kernel(tpc: int):
    import concourse.bass as bass
    import concourse.bacc as bacc
    import concourse.mybir as mybir
    from concourse.tile import TileContext

    f32 = mybir.dt.float32
    u32 = mybir.dt.uint32

    nc = bacc.Bacc()
    logits_d = nc.declare_dram_parameter("logits", [tpc, E], f32, isOutput=False)
    bias_d = nc.declare_dram_parameter("bias", [1, E], f32, isOutput=False)
    w_d = nc.declare_dram_parameter("weights", [tpc, K], f32, isOutput=True)
    i_d = nc.declare_dram_parameter("ids", [tpc, K], u32, isOutput=True)

    P = 128
    n_slab = tpc // P
    Sigmoid = mybir.ActivationFunctionType.Sigmoid
    Alu = mybir.AluOpType

    with TileContext(nc) as tc:
        with (
            tc.tile_pool(name="const", bufs=1) as const_pool,
            tc.tile_pool(name="big", bufs=3) as big,
            tc.tile_pool(name="small", bufs=4) as small,
            tc.tile_pool(name="out", bufs=4) as outp,
        ):
            bias_sb = const_pool.tile([P, E], f32)
            nc.gpsimd.dma_start(out=bias_sb, in_=bias_d[:].to_broadcast([P, E]))
            # pre-touch on DVE so later consumers carry at most one sync wait
            dummy = const_pool.tile([P, 1], f32)
            nc.vector.tensor_copy(out=dummy, in_=bias_sb[:, 0:1])

            for s in range(n_slab):
                t0 = s * P
                x = big.tile([P, E], f32, tag="x")
                nc.sync.dma_start(out=x, in_=logits_d[t0 : t0 + P, :])

                # match jax-on-neuron sigmoid bit-exactly: 1/(1+exp(-x))
                ex = big.tile([P, E], f32, tag="ex")
                nc.scalar.activation(
                    out=ex, in_=x, func=mybir.ActivationFunctionType.Exp, scale=-1.0
                )
                nc.scalar.add(out=ex, in_=ex, add=1.0)
                scores = big.tile([P, E], f32, tag="scores")
                nc.vector.reciprocal(out=scores, in_=ex)

                biased = big.tile([P, E], f32, tag="biased")
                nc.vector.tensor_tensor(
                    out=biased, in0=scores, in1=bias_sb, op=Alu.add
                )

                # --- group scores: top1 + top2 per group of 32 ---
                bg = biased.rearrange("p (g e) -> p g e", g=G)
                m1 = small.tile([P, G], f32, tag="m1")
                nc.vector.tensor_reduce(
                    out=m1, in_=bg, axis=mybir.AxisListType.X, op=Alu.max
                )
                rep = big.tile([P, E], f32, tag="rep")
                nc.vector.match_replace(
                    out=rep, in_to_replace=m1, in_values=biased, imm_value=NEG
                )
                m2 = small.tile([P, G], f32, tag="m2")
                nc.vector.tensor_reduce(
                    out=m2,
                    in_=rep.rearrange("p (g e) -> p g e", g=G),
                    axis=mybir.AxisListType.X,
                    op=Alu.max,
                )
                gs = small.tile([P, G], f32, tag="gs")
                nc.vector.tensor_tensor(out=gs, in0=m1, in1=m2, op=Alu.add)

                # --- select top-4 groups: threshold at 4th largest ---
                g8 = small.tile([P, 8], f32, tag="g8")
                nc.vector.max(out=g8, in_=gs)
                # neg[g] = (gs[g] < t) * NEG   (0 for kept groups)
                neg = small.tile([P, G], f32, tag="neg")
                nc.vector.tensor_scalar(
                    out=neg,
                    in0=gs,
                    scalar1=g8[:, 3:4],
                    scalar2=NEG,
                    op0=Alu.is_lt,
                    op1=Alu.mult,
                )
                masked = big.tile([P, E], f32, tag="masked")
                nc.vector.tensor_tensor(
                    out=masked,
                    in0=biased,
                    in1=neg.unsqueeze(2).to_broadcast([P, G, EPG]),
                    op=Alu.add,
                )

                # --- top-8 of masked biased: values + expert ids ---
                vals8 = small.tile([P, K], f32, tag="vals8")
                nc.vector.max(out=vals8, in_=masked)
                idx8 = small.tile([P, K], u32, tag="idx8")
                nc.vector.max_index(out=idx8, in_max=vals8, in_values=masked)

                # --- gather scores at the top-8 positions ---
                # indicator of the 8 winning positions
                ind = big.tile([P, E], f32, tag="ind")
                nc.vector.tensor_scalar(
                    out=ind,
                    in0=masked,
                    scalar1=vals8[:, 7:8],
                    scalar2=None,
                    op0=Alu.is_ge,
                )
                sel = big.tile([P, E], f32, tag="sel")
                nc.vector.tensor_tensor(out=sel, in0=scores, in1=ind, op=Alu.mult)
                s8 = small.tile([P, K], f32, tag="s8")
                nc.vector.max(out=s8, in_=sel)
                sidx8 = small.tile([P, K], u32, tag="sidx8")
                nc.vector.max_index(out=sidx8, in_max=s8, in_values=sel)

                # --- associate score-sorted (s8, sidx8) to rank order idx8 ---
                # C[p,k,j] = (idx8[p,k] == sidx8[p,j]); w8[p,k] = sum_j C*s8[p,j]
                idx8f = small.tile([P, K], f32, tag="idx8f")
                nc.scalar.copy(out=idx8f, in_=idx8)
                sidx8f = small.tile([P, K], f32, tag="sidx8f")
                nc.scalar.copy(out=sidx8f, in_=sidx8)
                cmat = small.tile([P, K, K], f32, tag="cmat")
                nc.vector.tensor_tensor(
                    out=cmat,
                    in0=idx8f.unsqueeze(2).to_broadcast([P, K, K]),
                    in1=sidx8f.unsqueeze(1).to_broadcast([P, K, K]),
                    op=Alu.is_equal,
                )
                w64 = small.tile([P, K, K], f32, tag="w64")
                nc.vector.tensor_tensor(
                    out=w64,
                    in0=cmat,
                    in1=s8.unsqueeze(1).to_broadcast([P, K, K]),
                    op=Alu.mult,
                )
                w8 = outp.tile([P, K], f32, tag="w8")
                nc.vector.tensor_reduce(
                    out=w8, in_=w64, axis=mybir.AxisListType.X, op=Alu.add
                )

                # --- renormalize: w * SCALE / (sum + 1e-20) ---
                wsum = small.tile([P, 1], f32, tag="wsum")
                nc.vector.tensor_reduce(
                    out=wsum, in_=w8, axis=mybir.AxisListType.X, op=Alu.add
                )
                nc.vector.tensor_scalar(
                    out=wsum,
                    in0=wsum,
                    scalar1=1.0e-20,
                    scalar2=None,
                    op0=Alu.add,
                )
                rcp = small.tile([P, 1], f32, tag="rcp")
                nc.vector.reciprocal(out=rcp, in_=wsum)
                nc.vector.tensor_scalar(
                    out=rcp,
                    in0=rcp,
                    scalar1=SCALE,
                    scalar2=None,
                    op0=Alu.mult,
                )
                wout = outp.tile([P, K], f32, tag="wout")
                nc.vector.tensor_scalar(
                    out=wout,
                    in0=w8,
                    scalar1=rcp,
                    scalar2=None,
                    op0=Alu.mult,
                )

                ids_out = outp.tile([P, K], u32, tag="ids_out")
                nc.vector.tensor_copy(out=ids_out, in_=idx8)

                nc.sync.dma_start(out=w_d[t0 : t0 + P, :], in_=wout)
                nc.sync.dma_start(out=i_d[t0 : t0 + P, :], in_=ids_out)

    nc.finalize()
    return nc


_NC_CACHE = {}


def _get_nc(tpc: int):
    if tpc not in _NC_CACHE:
        _NC_CACHE[tpc] = build_kernel(tpc)
    return _NC_CACHE[tpc]


def kernel(router_logits: np.ndarray, expert_bias: np.ndarray, _trace: bool = False):
    from concourse.bass_utils import run_bass_kernel_spmd

    router_logits = np.asarray(router_logits, dtype=np.float32)
    expert_bias = np.asarray(expert_bias, dtype=np.float32)
    tokens = router_logits.shape[0]
    assert tokens % N_CORES == 0
    tpc = tokens // N_CORES

    nc = _get_nc(tpc)
    bias_in = expert_bias.reshape(1, E)
    in_maps = [
        {
            "logits": np.ascontiguousarray(
                router_logits[c * tpc : (c + 1) * tpc]
            ),
            "bias": bias_in,
        }
        for c in range(N_CORES)
    ]
    res = run_bass_kernel_spmd(
        nc, in_maps, core_ids=list(range(N_CORES)), trace=_trace
    )
    weights = np.concatenate([r["weights"] for r in res.results], axis=0)
    ids = np.concatenate([r["ids"] for r in res.results], axis=0).astype(np.int32)
    if _trace:
        kernel.last_exec_time_ns = res.exec_time_ns
        kernel.last_mean_exec_time_ns = res.mean_exec_time_ns
    return weights, ids



# revision 8
# speedup vs baseline: 2.2540x; 2.2540x over previous
"""Group-limited MoE router kernel for Trainium2 (Bass/Tile), 8-core SPMD.

Per token (row of 256 experts):
  scores = sigmoid(logits); biased = scores + bias
  group_score[g] = top2sum(biased[g*32:(g+1)*32]) for 8 groups
  keep top-4 groups, mask the rest; topk_ids = top-8 of masked biased
  weights = scores[topk_ids] renormalized to sum 1, * 2.5

Strategy (v2): quantize biased to a positive int grid (2^-14 cells) and
pack an 8-bit score approximation into the low byte:
  ival   = int(scores*2^14 + bias*2^14 + 2^15)        in [22938, 58168]
  packed = ival*256 + scores*253                      exact int < 2^24 in fp32
Group top-2 via grouped-max / match_replace / grouped-max on ival.
Final top-8 via max8 + find_index8 on masked packed: positions give the
expert ids exactly; low 8 bits of the values give the scores for the
renormalized weights (score scale cancels in the renorm).

Engine split: ScalarE does sigmoid/score-scale/hi-extract; GpSimd does the
three elementwise passes (ival, packed, mask-apply); VectorE does the
reduces, match_replace, max8/find_index8 and small glue.

Data-parallel over tokens: 131072 -> 8 cores x 16384; 128 tokens per
partition-slab, S=4 slabs batched per instruction block.
"""

import numpy as np

TOKENS = 131072
E = 256
G = 8
EPG = 32
K = 8
N_CORES = 8

P = 128
S = 4  # slabs per instruction block

IV_SCALE = 16384.0  # 2^14 quantization of biased
IV_OFF = 32768.0  # keep ival strictly positive
MAGIC = 12582912.0  # 3*2^22: float add forces round-to-int in [2^23, 2^24)
SQ_SCALE = 253.0  # score packed into low byte (253 leaves rounding margin)
NEGP = -33554432.0  # -2^25 group mask in packed domain
MATCH_IMM = -1.0  # replaces group maxima (all ival > 0)
HI_SCALE = 1.0 / 256.0
HI_BIAS = -0.494  # centers sq/256-0.494 in (-0.5, 0.5) for round-nearest
WSUM_PRE = 1.0 / 2.5  # w = sq / (0.4 * sum(sq)) == 2.5 * score / sum(score)


def build_kernel(tpc: int):
    import concourse.bass as bass
    import concourse.bacc as bacc
    import concourse.mybir as mybir
    from concourse.tile import TileContext

    f32 = mybir.dt.float32
    i32 = mybir.dt.int32
    u32 = mybir.dt.uint32
    Alu = mybir.AluOpType
    Act = mybir.ActivationFunctionType
    X = mybir.AxisListType.X

    nc = bacc.Bacc()
    logits_d = nc.declare_dram_parameter("logits", [tpc, E], f32, isOutput=False)
    # host precomputes biasq = bias*2^14 + 2^15 + 3*2^22 (magic rounder)
    biasq_d = nc.declare_dram_parameter("biasq", [1, E], f32, isOutput=False)
    w_d = nc.declare_dram_parameter("weights", [tpc, K], f32, isOutput=True)
    i_d = nc.declare_dram_parameter("ids", [tpc, K], u32, isOutput=True)

    assert tpc % (P * S) == 0
    n_blk = tpc // (P * S)
    SE = S * E
    SG = S * G
    SK = S * K

    with TileContext(nc) as tc:
        with (
            tc.tile_pool(name="const", bufs=1) as cpool,
            tc.tile_pool(name="big", bufs=2) as big,
            tc.tile_pool(name="sm", bufs=3) as sm,
            tc.tile_pool(name="out", bufs=3) as outp,
        ):
            biasq = cpool.tile([P, E], f32)
            nc.gpsimd.dma_start(out=biasq, in_=biasq_d[:].to_broadcast([P, E]))
            # pre-touch so consumers don't each wait on the DMA
            dummy = cpool.tile([P, 1], f32)
            nc.vector.tensor_copy(out=dummy, in_=biasq[:, 0:1])

            for b in range(n_blk):
                t0 = b * P * S
                x = big.tile([P, SE], f32, tag="x")
                nc.sync.dma_start(
                    out=x.rearrange("p (s e) -> p s e", e=E),
                    in_=logits_d[t0 : t0 + S * P, :].rearrange(
                        "(s p) e -> p s e", p=P
                    ),
                )

                scores = big.tile([P, SE], f32, tag="scores")
                nc.scalar.activation(out=scores, in_=x, func=Act.Sigmoid)
                sq8 = big.tile([P, SE], f32, tag="sq8")
                nc.scalar.activation(out=sq8, in_=scores, func=Act.Copy, scale=SQ_SCALE)
                s2k = big.tile([P, SE], f32, tag="s2k")
                nc.scalar.activation(out=s2k, in_=scores, func=Act.Copy, scale=IV_SCALE)

                # ivalm = round(scores*2^14 + bias*2^14 + 2^15) + 3*2^22
                # (magic-number rounding: result lands in [2^23, 2^24) where
                #  fp32 ulp is 1, so the add itself quantizes)  [GpSimd]
                ivalm = big.tile([P, SE], f32, tag="ivalm")
                nc.gpsimd.tensor_tensor(
                    out=ivalm,
                    in0=s2k,
                    in1=biasq.unsqueeze(1).to_broadcast([P, S, E]),
                    op=Alu.add,
                )
                # iv256 = (ivalm - magic)*256, exact: ivalm*256 is an exponent
                # shift and 3*2^30 cancels without rounding  [GpSimd]
                iv256 = big.tile([P, SE], f32, tag="iv256")
                nc.gpsimd.tensor_scalar(
                    out=iv256,
                    in0=ivalm,
                    scalar1=256.0,
                    scalar2=-MAGIC * 256.0,
                    op0=Alu.mult,
                    op1=Alu.add,
                )
                packed = big.tile([P, SE], f32, tag="packed")
                nc.gpsimd.tensor_tensor(out=packed, in0=iv256, in1=sq8, op=Alu.add)

                # group top-2 on ivalm [VectorE]
                m1 = sm.tile([P, SG], f32, tag="m1")
                nc.vector.tensor_reduce(
                    out=m1,
                    in_=ivalm.rearrange("p (sg e) -> p sg e", e=EPG),
                    axis=X,
                    op=Alu.max,
                )
                rep = big.tile([P, SE], f32, tag="rep")
                for s in range(S):
                    nc.vector.match_replace(
                        out=rep[:, s * E : (s + 1) * E],
                        in_to_replace=m1[:, s * G : (s + 1) * G],
                        in_values=ivalm[:, s * E : (s + 1) * E],
                        imm_value=MATCH_IMM,
                    )
                m2 = sm.tile([P, SG], f32, tag="m2")
                nc.vector.tensor_reduce(
                    out=m2,
                    in_=rep.rearrange("p (sg e) -> p sg e", e=EPG),
                    axis=X,
                    op=Alu.max,
                )
                gs = sm.tile([P, SG], f32, tag="gs")
                nc.vector.tensor_tensor(out=gs, in0=m1, in1=m2, op=Alu.add)

                # top-4 groups -> additive mask in packed domain
                g8 = sm.tile([P, SG], f32, tag="g8")
                for s in range(S):
                    sl = slice(s * G, (s + 1) * G)
                    nc.vector.max(out=g8[:, sl], in_=gs[:, sl])
                # ind = (gs < 4th-largest of slab), batched via strided t4 view
                negp = sm.tile([P, SG], f32, tag="negp")
                nc.vector.tensor_tensor(
                    out=negp.rearrange("p (s g) -> p s g", g=G),
                    in0=gs.rearrange("p (s g) -> p s g", g=G),
                    in1=g8[:, 3::K].unsqueeze(2).to_broadcast([P, S, G]),
                    op=Alu.is_lt,
                )
                nc.vector.tensor_scalar(
                    out=negp,
                    in0=negp,
                    scalar1=NEGP,
                    scalar2=None,
                    op0=Alu.mult,
                )

                maskedP = big.tile([P, SE], f32, tag="maskedP")
                nc.gpsimd.tensor_tensor(
                    out=maskedP,
                    in0=packed,
                    in1=negp.unsqueeze(2).to_broadcast([P, SG, EPG]),
                    op=Alu.add,
                )

                # final top-8: values (scores in low byte) + positions (ids)
                p8 = sm.tile([P, SK], f32, tag="p8")
                idx8 = outp.tile([P, SK], u32, tag="idx8")
                for s in range(S):
                    sl = slice(s * K, (s + 1) * K)
                    nc.vector.max(out=p8[:, sl], in_=maskedP[:, s * E : (s + 1) * E])
                    nc.vector.max_index(
                        out=idx8[:, sl],
                        in_max=p8[:, sl],
                        in_values=maskedP[:, s * E : (s + 1) * E],
                    )

                # sq = p8 - 256*round(p8/256 - 0.494); weights = sq/(0.4*sum(sq))
                # round() via magic-number add (mode-independent, RNE fp add):
                # him = p8/256 - 0.494 + 3*2^22 rounds to ival + 3*2^22
                him = sm.tile([P, SK], f32, tag="him")
                nc.scalar.activation(
                    out=him, in_=p8, func=Act.Copy, scale=HI_SCALE, bias=HI_BIAS
                )
                hif = sm.tile([P, SK], f32, tag="hif")
                nc.vector.tensor_scalar(
                    out=hif,
                    in0=him,
                    scalar1=MAGIC,
                    scalar2=-MAGIC,
                    op0=Alu.add,
                    op1=Alu.add,
                )
                sqv = sm.tile([P, SK], f32, tag="sqv")
                nc.vector.scalar_tensor_tensor(
                    out=sqv,
                    in0=hif,
                    scalar=-256.0,
                    in1=p8,
                    op0=Alu.mult,
                    op1=Alu.add,
                )
                wsum = sm.tile([P, S], f32, tag="wsum")
                nc.vector.tensor_reduce(
                    out=wsum,
                    in_=sqv.rearrange("p (s k) -> p s k", k=K),
                    axis=X,
                    op=Alu.add,
                )
                nc.scalar.activation(
                    out=wsum, in_=wsum, func=Act.Copy, scale=WSUM_PRE
                )
                rcp = sm.tile([P, S], f32, tag="rcp")
                nc.vector.reciprocal(out=rcp, in_=wsum)

                wout = outp.tile([P, SK], f32, tag="wout")
                nc.vector.tensor_tensor(
                    out=wout.rearrange("p (s k) -> p s k", k=K),
                    in0=sqv.rearrange("p (s k) -> p s k", k=K),
                    in1=rcp.unsqueeze(2).to_broadcast([P, S, K]),
                    op=Alu.mult,
                )

                rows = slice(t0, t0 + S * P)
                nc.sync.dma_start(
                    out=w_d[rows, :].rearrange("(s p) k -> p s k", p=P),
                    in_=wout.rearrange("p (s k) -> p s k", k=K),
                )
                nc.sync.dma_start(
                    out=i_d[rows, :].rearrange("(s p) k -> p s k", p=P),
                    in_=idx8.rearrange("p (s k) -> p s k", k=K),
                )

    nc.finalize()
    return nc


_NC_CACHE = {}


def _get_nc(tpc: int):
    if tpc not in _NC_CACHE:
        _NC_CACHE[tpc] = build_kernel(tpc)
    return _NC_CACHE[tpc]


def kernel(router_logits: np.ndarray, expert_bias: np.ndarray, _trace: bool = False):
    from concourse.bass_utils import run_bass_kernel_spmd

    router_logits = np.asarray(router_logits, dtype=np.float32)
    expert_bias = np.asarray(expert_bias, dtype=np.float32)
    tokens = router_logits.shape[0]
    assert tokens % N_CORES == 0
    tpc = tokens // N_CORES

    nc = _get_nc(tpc)
    biasq = (expert_bias.astype(np.float64) * IV_SCALE + IV_OFF + MAGIC).astype(
        np.float32
    ).reshape(1, E)
    in_maps = [
        {
            "logits": np.ascontiguousarray(router_logits[c * tpc : (c + 1) * tpc]),
            "biasq": biasq,
        }
        for c in range(N_CORES)
    ]
    res = run_bass_kernel_spmd(
        nc, in_maps, core_ids=list(range(N_CORES)), trace=_trace
    )
    weights = np.concatenate([r["weights"] for r in res.results], axis=0)
    ids = np.concatenate([r["ids"] for r in res.results], axis=0).astype(np.int32)
    if _trace:
        kernel.last_exec_time_ns = res.exec_time_ns
        kernel.last_mean_exec_time_ns = res.mean_exec_time_ns
    return weights, ids


# revision 9
# speedup vs baseline: 2.2725x; 1.0082x over previous
"""Group-limited MoE router kernel for Trainium2 (Bass/Tile), 8-core SPMD.

Per token (row of 256 experts):
  scores = sigmoid(logits); biased = scores + bias
  group_score[g] = top2sum(biased[g*32:(g+1)*32]) for 8 groups
  keep top-4 groups, mask the rest; topk_ids = top-8 of masked biased
  weights = scores[topk_ids] renormalized to sum 1, * 2.5

Strategy (v2): quantize biased to a positive int grid (2^-14 cells) and
pack an 8-bit score approximation into the low byte:
  ival   = int(scores*2^14 + bias*2^14 + 2^15)        in [22938, 58168]
  packed = ival*256 + scores*253                      exact int < 2^24 in fp32
Group top-2 via grouped-max / match_replace / grouped-max on ival.
Final top-8 via max8 + find_index8 on masked packed: positions give the
expert ids exactly; low 8 bits of the values give the scores for the
renormalized weights (score scale cancels in the renorm).

Engine split: ScalarE does sigmoid/score-scale/hi-extract; GpSimd does the
three elementwise passes (ival, packed, mask-apply); VectorE does the
reduces, match_replace, max8/find_index8 and small glue.

Data-parallel over tokens: 131072 -> 8 cores x 16384; 128 tokens per
partition-slab, S=4 slabs batched per instruction block.
"""

import numpy as np

TOKENS = 131072
E = 256
G = 8
EPG = 32
K = 8
N_CORES = 8

P = 128
S = 4  # slabs per instruction block

IV_SCALE = 16384.0  # 2^14 quantization of biased
IV_OFF = 32768.0  # keep ival strictly positive
MAGIC = 12582912.0  # 3*2^22: float add forces round-to-int in [2^23, 2^24)
SQ_SCALE = 253.0  # score packed into low byte (253 leaves rounding margin)
NEGP = -33554432.0  # -2^25 group mask in packed domain
MATCH_IMM = -1.0  # replaces group maxima (all ival > 0)
HI_SCALE = 1.0 / 256.0
HI_BIAS = -0.494  # centers sq/256-0.494 in (-0.5, 0.5) for round-nearest
WSUM_PRE = 1.0 / 2.5  # w = sq / (0.4 * sum(sq)) == 2.5 * score / sum(score)


def build_kernel(tpc: int):
    import concourse.bass as bass
    import concourse.bacc as bacc
    import concourse.mybir as mybir
    from concourse.tile import TileContext

    f32 = mybir.dt.float32
    i32 = mybir.dt.int32
    u32 = mybir.dt.uint32
    Alu = mybir.AluOpType
    Act = mybir.ActivationFunctionType
    X = mybir.AxisListType.X

    nc = bacc.Bacc()
    logits_d = nc.declare_dram_parameter("logits", [tpc, E], f32, isOutput=False)
    # host precomputes biasq = bias*2^14 + 2^15 + 3*2^22 (magic rounder)
    biasq_d = nc.declare_dram_parameter("biasq", [1, E], f32, isOutput=False)
    w_d = nc.declare_dram_parameter("weights", [tpc, K], f32, isOutput=True)
    i_d = nc.declare_dram_parameter("ids", [tpc, K], u32, isOutput=True)

    assert tpc % (P * S) == 0
    n_blk = tpc // (P * S)
    SE = S * E
    SG = S * G
    SK = S * K

    with TileContext(nc) as tc:
        with (
            tc.tile_pool(name="const", bufs=1) as cpool,
            tc.tile_pool(name="big", bufs=3) as big,
            tc.tile_pool(name="sm", bufs=3) as sm,
            tc.tile_pool(name="out", bufs=3) as outp,
        ):
            biasq = cpool.tile([P, E], f32)
            nc.gpsimd.dma_start(out=biasq, in_=biasq_d[:].to_broadcast([P, E]))
            # pre-touch so consumers don't each wait on the DMA
            dummy = cpool.tile([P, 1], f32)
            nc.vector.tensor_copy(out=dummy, in_=biasq[:, 0:1])

            for b in range(n_blk):
                t0 = b * P * S
                x = big.tile([P, SE], f32, tag="x")
                nc.sync.dma_start(
                    out=x.rearrange("p (s e) -> p s e", e=E),
                    in_=logits_d[t0 : t0 + S * P, :].rearrange(
                        "(s p) e -> p s e", p=P
                    ),
                )

                scores = big.tile([P, SE], f32, tag="scores")
                nc.scalar.activation(out=scores, in_=x, func=Act.Sigmoid)
                sq8 = big.tile([P, SE], f32, tag="sq8")
                nc.scalar.activation(out=sq8, in_=scores, func=Act.Copy, scale=SQ_SCALE)
                s2k = big.tile([P, SE], f32, tag="s2k")
                nc.scalar.activation(out=s2k, in_=scores, func=Act.Copy, scale=IV_SCALE)

                # ivalm = round(scores*2^14 + bias*2^14 + 2^15) + 3*2^22
                # (magic-number rounding: result lands in [2^23, 2^24) where
                #  fp32 ulp is 1, so the add itself quantizes)  [GpSimd]
                ivalm = big.tile([P, SE], f32, tag="ivalm")
                nc.gpsimd.tensor_tensor(
                    out=ivalm,
                    in0=s2k,
                    in1=biasq.unsqueeze(1).to_broadcast([P, S, E]),
                    op=Alu.add,
                )
                # iv256 = (ivalm - magic)*256, exact: ivalm*256 is an exponent
                # shift and 3*2^30 cancels without rounding  [ScalarE]
                iv256 = big.tile([P, SE], f32, tag="iv256")
                nc.scalar.activation(
                    out=iv256,
                    in_=ivalm,
                    func=Act.Copy,
                    scale=256.0,
                    bias=-MAGIC * 256.0,
                )
                packed = big.tile([P, SE], f32, tag="packed")
                nc.gpsimd.tensor_tensor(out=packed, in0=iv256, in1=sq8, op=Alu.add)

                # group top-2 on ivalm [VectorE]
                m1 = sm.tile([P, SG], f32, tag="m1")
                nc.vector.tensor_reduce(
                    out=m1,
                    in_=ivalm.rearrange("p (sg e) -> p sg e", e=EPG),
                    axis=X,
                    op=Alu.max,
                )
                rep = big.tile([P, SE], f32, tag="rep")
                for s in range(S):
                    nc.vector.match_replace(
                        out=rep[:, s * E : (s + 1) * E],
                        in_to_replace=m1[:, s * G : (s + 1) * G],
                        in_values=ivalm[:, s * E : (s + 1) * E],
                        imm_value=MATCH_IMM,
                    )
                m2 = sm.tile([P, SG], f32, tag="m2")
                nc.vector.tensor_reduce(
                    out=m2,
                    in_=rep.rearrange("p (sg e) -> p sg e", e=EPG),
                    axis=X,
                    op=Alu.max,
                )
                gs = sm.tile([P, SG], f32, tag="gs")
                nc.vector.tensor_tensor(out=gs, in0=m1, in1=m2, op=Alu.add)

                # top-4 groups via rank count: drop g if #{j: gs_j >= gs_g} > 4
                cmp = sm.tile([P, SG * G], f32, tag="cmp")
                gs3 = gs.rearrange("p (s g) -> p s g", g=G)
                nc.vector.tensor_tensor(
                    out=cmp.rearrange("p (s i j) -> p s i j", i=G, j=G),
                    in0=gs3.unsqueeze(3).to_broadcast([P, S, G, G]),
                    in1=gs3.unsqueeze(2).to_broadcast([P, S, G, G]),
                    op=Alu.is_le,
                )
                cnt = sm.tile([P, SG], f32, tag="cnt")
                nc.vector.tensor_reduce(
                    out=cnt,
                    in_=cmp.rearrange("p (sg j) -> p sg j", j=G),
                    axis=X,
                    op=Alu.add,
                )
                negp = sm.tile([P, SG], f32, tag="negp")
                nc.vector.tensor_scalar(
                    out=negp,
                    in0=cnt,
                    scalar1=4.5,
                    scalar2=NEGP,
                    op0=Alu.is_gt,
                    op1=Alu.mult,
                )

                maskedP = big.tile([P, SE], f32, tag="maskedP")
                nc.gpsimd.tensor_tensor(
                    out=maskedP,
                    in0=packed,
                    in1=negp.unsqueeze(2).to_broadcast([P, SG, EPG]),
                    op=Alu.add,
                )

                # final top-8: values (scores in low byte) + positions (ids)
                p8 = sm.tile([P, SK], f32, tag="p8")
                idx8 = outp.tile([P, SK], u32, tag="idx8")
                for s in range(S):
                    sl = slice(s * K, (s + 1) * K)
                    nc.vector.max(out=p8[:, sl], in_=maskedP[:, s * E : (s + 1) * E])
                    nc.vector.max_index(
                        out=idx8[:, sl],
                        in_max=p8[:, sl],
                        in_values=maskedP[:, s * E : (s + 1) * E],
                    )

                # sq = p8 - 256*round(p8/256 - 0.494); weights = sq/(0.4*sum(sq))
                # round() via magic-number add (mode-independent, RNE fp add):
                # him = p8/256 - 0.494 + 3*2^22 rounds to ival + 3*2^22
                him = sm.tile([P, SK], f32, tag="him")
                nc.scalar.activation(
                    out=him, in_=p8, func=Act.Copy, scale=HI_SCALE, bias=HI_BIAS
                )
                hif = sm.tile([P, SK], f32, tag="hif")
                nc.vector.tensor_scalar(
                    out=hif,
                    in0=him,
                    scalar1=MAGIC,
                    scalar2=-MAGIC,
                    op0=Alu.add,
                    op1=Alu.add,
                )
                sqv = sm.tile([P, SK], f32, tag="sqv")
                nc.vector.scalar_tensor_tensor(
                    out=sqv,
                    in0=hif,
                    scalar=-256.0,
                    in1=p8,
                    op0=Alu.mult,
                    op1=Alu.add,
                )
                wsum = sm.tile([P, S], f32, tag="wsum")
                nc.vector.tensor_reduce(
                    out=wsum,
                    in_=sqv.rearrange("p (s k) -> p s k", k=K),
                    axis=X,
                    op=Alu.add,
                )
                nc.scalar.activation(
                    out=wsum, in_=wsum, func=Act.Copy, scale=WSUM_PRE
                )
                rcp = sm.tile([P, S], f32, tag="rcp")
                nc.vector.reciprocal(out=rcp, in_=wsum)

                wout = outp.tile([P, SK], f32, tag="wout")
                nc.vector.tensor_tensor(
                    out=wout.rearrange("p (s k) -> p s k", k=K),
                    in0=sqv.rearrange("p (s k) -> p s k", k=K),
                    in1=rcp.unsqueeze(2).to_broadcast([P, S, K]),
                    op=Alu.mult,
                )

                rows = slice(t0, t0 + S * P)
                nc.sync.dma_start(
                    out=w_d[rows, :].rearrange("(s p) k -> p s k", p=P),
                    in_=wout.rearrange("p (s k) -> p s k", k=K),
                )
                nc.sync.dma_start(
                    out=i_d[rows, :].rearrange("(s p) k -> p s k", p=P),
                    in_=idx8.rearrange("p (s k) -> p s k", k=K),
                )

    nc.finalize()
    return nc


_NC_CACHE = {}


def _get_nc(tpc: int):
    if tpc not in _NC_CACHE:
        _NC_CACHE[tpc] = build_kernel(tpc)
    return _NC_CACHE[tpc]


def kernel(router_logits: np.ndarray, expert_bias: np.ndarray, _trace: bool = False):
    from concourse.bass_utils import run_bass_kernel_spmd

    router_logits = np.asarray(router_logits, dtype=np.float32)
    expert_bias = np.asarray(expert_bias, dtype=np.float32)
    tokens = router_logits.shape[0]
    assert tokens % N_CORES == 0
    tpc = tokens // N_CORES

    nc = _get_nc(tpc)
    biasq = (expert_bias.astype(np.float64) * IV_SCALE + IV_OFF + MAGIC).astype(
        np.float32
    ).reshape(1, E)
    in_maps = [
        {
            "logits": np.ascontiguousarray(router_logits[c * tpc : (c + 1) * tpc]),
            "biasq": biasq,
        }
        for c in range(N_CORES)
    ]
    res = run_bass_kernel_spmd(
        nc, in_maps, core_ids=list(range(N_CORES)), trace=_trace
    )
    weights = np.concatenate([r["weights"] for r in res.results], axis=0)
    ids = np.concatenate([r["ids"] for r in res.results], axis=0).astype(np.int32)
    if _trace:
        kernel.last_exec_time_ns = res.exec_time_ns
        kernel.last_mean_exec_time_ns = res.mean_exec_time_ns
    return weights, ids
